# revision 6
# baseline (speedup 1.0000x reference)
"""Kaldi fbank (torchaudio.compliance.kaldi defaults, 80 mel bins) on 8
Trainium2 NeuronCores via Bass/Tile.

Strategy: every pre-FFT step (framing -> DC removal -> preemphasis -> Povey
window) is linear in the frame, so the whole frame->spectrum map folds into
two constant matrices G_re/G_im [400, 256] (Nyquist bin 256 dropped: its mel
weight is exactly zero).  Per frame: power = (f@G_re)^2 + (f@G_im)^2, then
mel = power @ W^T, out = log(max(mel, eps)).  All heavy work is tensor-engine
matmuls in float32r (full-rate fp32 path, ~10-bit mantissa operands).

Sharding: batch 32 -> 8 cores x 4 waveforms (embarrassingly data-parallel).
"""

import numpy as np

SR = 16000
WIN = 400
SHIFT = 160
NFFT = 512
NMEL = 80
PREEMPH = 0.97
EPS = 1.1920929e-07

B_FULL = 32
L = 160000
N_CORES = 8
B_CORE = B_FULL // N_CORES          # 4 waveforms per core
M_FRAMES = 1 + (L - WIN) // SHIFT   # 998
NJ = L // SHIFT                     # 1000 blocks of 160 samples
NFREQ = 256                         # bins 0..255 (bin 256 has zero mel weight)

# frame blocks (moving-operand N per matmul; fp32 max is 512)
FRAME_BLOCKS = [(0, 512), (512, M_FRAMES - 512)]
# K chunking of the 400-sample window into 4 partition chunks
K_SIZES = [128, 128, 128, 16]


def _build_consts():
    """G_re/G_im [400, 256] and mel weights [256, 80], fp64 math -> fp32."""
    t = np.arange(WIN, dtype=np.float64)
    povey = (0.5 - 0.5 * np.cos(2.0 * np.pi * t / (WIN - 1))) ** 0.85
    M1 = np.eye(WIN) - np.ones((WIN, WIN)) / WIN      # remove_dc_offset
    P = np.eye(WIN)
    P[0, 0] = 1.0 - PREEMPH                            # preemphasis (replicate pad)
    for i in range(1, WIN):
        P[i, i - 1] = -PREEMPH
    A = povey[:, None] * (P @ M1)                      # [400, 400] combined linear map
    u = np.arange(WIN)[:, None]
    k = np.arange(NFREQ)[None, :]
    ang = 2.0 * np.pi * u * k / NFFT
    G_re = (A.T @ np.cos(ang)).astype(np.float32)      # [400, 256]
    G_im = (A.T @ -np.sin(ang)).astype(np.float32)

    def mel(f):
        return 1127.0 * np.log(1.0 + f / 700.0)

    fft_freqs = np.arange(NFFT // 2) * (SR / NFFT)
    m = mel(fft_freqs)
    ml, mh = mel(20.0), mel(8000.0)
    d = (mh - ml) / (NMEL + 1)
    left = ml + np.arange(NMEL)[:, None] * d
    center = left + d
    right = center + d
    w = np.maximum(0.0, np.minimum((m - left) / (center - left),
                                   (right - m) / (right - center)))  # [80, 256]
    MELW_T = np.ascontiguousarray(w.T).astype(np.float32)            # [256, 80]
    return G_re, G_im, MELW_T


def _build_bass():
    import concourse.mybir as mybir
    from concourse import bacc
    from concourse.masks import make_identity
    from concourse.tile import TileContext

    f32 = mybir.dt.float32
    f32r = mybir.dt.float32r

    nc = bacc.Bacc("TRN2", target_bir_lowering=False, debug=False,
                   num_devices=N_CORES)
    waves = nc.dram_tensor("waves", [B_CORE, L], f32, kind="ExternalInput").ap()
    gre_d = nc.dram_tensor("gre", [WIN, NFREQ], f32, kind="ExternalInput").ap()
    gim_d = nc.dram_tensor("gim", [WIN, NFREQ], f32, kind="ExternalInput").ap()
    melw_d = nc.dram_tensor("melw", [NFREQ, NMEL], f32, kind="ExternalInput").ap()
    out_d = nc.dram_tensor("out", [B_CORE, NMEL, M_FRAMES], f32,
                           kind="ExternalOutput").ap()

    with TileContext(nc) as tc:
        with (
            tc.tile_pool(name="consts", bufs=1) as cpool,
            tc.tile_pool(name="stage", bufs=2) as stpool,
            tc.tile_pool(name="w160", bufs=2) as wpool,
            tc.tile_pool(name="vload", bufs=3) as vpool,
            tc.tile_pool(name="work", bufs=2) as spool,
            tc.tile_pool(name="psum_t", bufs=2, space="PSUM") as pt,
            tc.tile_pool(name="psum_d", bufs=2, space="PSUM") as pd,
            tc.tile_pool(name="psum_m", bufs=2, space="PSUM") as pm,
        ):
            # ---- constants ----
            ident = cpool.tile([128, 128], f32, tag="ident")
            make_identity(nc, ident[:])

            # lhsT K-chunk tiles in f32r (DMA the fp32 bits to a staging
            # tile, then an ACT copy performs the f32r rounding walrus
            # requires of every producer feeding an FP32R matmul).
            # q3 (K=16) must share the rhs base partition 64, so its tile
            # is [80, .] with data in rows 64:80.
            def load_rounded(dst_ap, src_ap, rows, tagn):
                st = stpool.tile([rows, NFREQ], f32, tag="stage")
                nc.sync.dma_start(out=st[:], in_=src_ap)
                nc.scalar.copy(out=dst_ap, in_=st[:])

            gre_t, gim_t = [], []
            r0 = 0
            for q, ks in enumerate(K_SIZES):
                if q < 3:
                    gr = cpool.tile([ks, NFREQ], f32r, tag=f"gre{q}")
                    gi = cpool.tile([ks, NFREQ], f32r, tag=f"gim{q}")
                    load_rounded(gr[:], gre_d[r0:r0 + ks, :], ks, q)
                    load_rounded(gi[:], gim_d[r0:r0 + ks, :], ks, q)
                    gre_t.append(gr)
                    gim_t.append(gi)
                else:
                    gr_full = cpool.tile([80, NFREQ], f32r, tag=f"gre{q}")
                    gi_full = cpool.tile([80, NFREQ], f32r, tag=f"gim{q}")
                    load_rounded(gr_full[64:80, :], gre_d[r0:r0 + ks, :], ks, q)
                    load_rounded(gi_full[64:80, :], gim_d[r0:r0 + ks, :], ks, q)
                    gre_t.append(gr_full[64:80, :])
                    gim_t.append(gi_full[64:80, :])
                r0 += ks

            melw_t = []
            for c in range(2):
                mw = cpool.tile([128, NMEL], f32r, tag=f"melw{c}")
                st = stpool.tile([128, NMEL], f32, tag="stage_m")
                nc.sync.dma_start(out=st[:], in_=melw_d[c * 128:(c + 1) * 128, :])
                nc.scalar.copy(out=mw[:], in_=st[:])
                melw_t.append(mw)

            for b in range(B_CORE):
                wav_js = waves[b].rearrange("(j s) -> j s", s=SHIFT)  # [1000, 160]

                # ---- phase T: build W160[s, j] = wave[160 j + s] ----
                # (f32r tiles; the PSUM->SBUF copies do the rounding)
                wtop = wpool.tile([128, NJ], f32r, tag="wtop")   # s in [0, 128)
                wbot = wpool.tile([32, NJ], f32r, tag="wbot")    # s in [128, 160)
                for c in range(8):
                    j0 = c * 128
                    p_c = min(128, NJ - j0)                      # 128 or 104
                    v = vpool.tile([p_c, SHIFT], f32, tag="v")
                    nc.sync.dma_start(out=v[:], in_=wav_js[j0:j0 + p_c, :])
                    tp0 = pt.tile([128, p_c], f32, tag="tp")
                    nc.tensor.transpose(tp0[:], v[:, 0:128], ident[:p_c, :p_c])
                    nc.vector.tensor_copy(wtop[:, j0:j0 + p_c], tp0[:])
                    tp1 = pt.tile([32, p_c], f32, tag="tp")
                    nc.tensor.transpose(tp1[:], v[:, 128:160], ident[:p_c, :p_c])
                    nc.vector.tensor_copy(wbot[:, j0:j0 + p_c], tp1[:])

                # K-chunk rhs tensors.  t in [128,256): rows 0:32 = wbot@j,
                # rows 32:128 = wtop[0:96]@(j+1).  t in [256,384): wtop[96:128]@(j+1),
                # wbot[0:32]@(j+1), wtop[0:64]@(j+2).  t in [384,400): wtop[64:80]@(j+2).
                # (gpsimd APs must be 32-aligned 32/64/128-partition blocks)
                wmid = wpool.tile([128, NJ], f32r, tag="wmid")
                nc.gpsimd.tensor_copy(wmid[0:32, 0:NJ], wbot[0:32, 0:NJ])
                nc.gpsimd.tensor_copy(wmid[32:64, 0:NJ - 1], wtop[0:32, 1:NJ])
                nc.gpsimd.tensor_copy(wmid[64:96, 0:NJ - 1], wtop[32:64, 1:NJ])
                nc.gpsimd.tensor_copy(wmid[96:128, 0:NJ - 1], wtop[64:96, 1:NJ])
                wmid2 = wpool.tile([128, NJ], f32r, tag="wmid2")
                nc.gpsimd.tensor_copy(wmid2[0:32, 0:NJ - 1], wtop[96:128, 1:NJ])
                nc.gpsimd.tensor_copy(wmid2[32:64, 0:NJ - 1], wbot[0:32, 1:NJ])
                nc.gpsimd.tensor_copy(wmid2[64:96, 0:NJ - 2], wtop[0:32, 2:NJ])
                nc.gpsimd.tensor_copy(wmid2[96:128, 0:NJ - 2], wtop[32:64, 2:NJ])

                # ---- phases D + M per frame block ----
                for (i0, nfb) in FRAME_BLOCKS:
                    rhs = [
                        wtop[0:128, i0:i0 + nfb],
                        wmid[0:128, i0:i0 + nfb],
                        wmid2[0:128, i0:i0 + nfb],
                        wtop[64:80, i0 + 2:i0 + 2 + nfb],
                    ]
                    power = []
                    for mi in range(2):
                        mlo, mhi = mi * 128, (mi + 1) * 128
                        ps_re = pd.tile([128, nfb], f32, tag="ps_re")
                        ps_im = pd.tile([128, nfb], f32, tag="ps_im")
                        for q in range(4):
                            nc.tensor.matmul(
                                ps_re[:], gre_t[q][:, mlo:mhi], rhs[q],
                                start=(q == 0), stop=(q == 3))
                        for q in range(4):
                            nc.tensor.matmul(
                                ps_im[:], gim_t[q][:, mlo:mhi], rhs[q],
                                start=(q == 0), stop=(q == 3))
                        sq = spool.tile([128, nfb], f32, tag="sq")
                        nc.scalar.square(sq[:], ps_re[:])
                        sq2 = spool.tile([128, nfb], f32, tag="sq2")
                        nc.scalar.square(sq2[:], ps_im[:])
                        pw = spool.tile([128, nfb], f32r, tag="pw")
                        nc.vector.tensor_add(pw[:], sq[:], sq2[:])
                        power.append(pw)

                    ps_mel = pm.tile([NMEL, nfb], f32, tag="mel")
                    for mi in range(2):
                        nc.tensor.matmul(
                            ps_mel[:], melw_t[mi][:], power[mi][:],
                            start=(mi == 0), stop=(mi == 1))
                    mel_sb = spool.tile([NMEL, nfb], f32, tag="mel_sb")
                    nc.vector.tensor_scalar_max(mel_sb[:], ps_mel[:], EPS)
                    out_sb = spool.tile([NMEL, nfb], f32, tag="out_sb")
                    nc.scalar.activation(out_sb[:], mel_sb[:],
                                         mybir.ActivationFunctionType.Ln)
                    nc.sync.dma_start(out=out_d[b][:, i0:i0 + nfb], in_=out_sb[:])

    nc.compile()
    return nc


_CACHE = {}


def kernel(waveforms) -> np.ndarray:
    from concourse.bass_utils import run_bass_kernel_spmd

    w = np.ascontiguousarray(np.asarray(waveforms, dtype=np.float32))
    assert w.shape == (B_FULL, L), w.shape

    if "nc" not in _CACHE:
        _CACHE["consts"] = _build_consts()
        _CACHE["nc"] = _build_bass()
    G_re, G_im, MELW_T = _CACHE["consts"]
    nc = _CACHE["nc"]

    shards = w.reshape(N_CORES, B_CORE, L)
    in_maps = [
        {"waves": shards[c], "gre": G_re, "gim": G_im, "melw": MELW_T}
        for c in range(N_CORES)
    ]
    r = run_bass_kernel_spmd(nc, in_maps, list(range(N_CORES)))
    out = np.concatenate([r.results[c]["out"] for c in range(N_CORES)], axis=0)
    return out


# revision 8
# speedup vs baseline: 1.0048x; 1.0048x over previous
"""Kaldi fbank (torchaudio.compliance.kaldi defaults, 80 mel bins) on 8
Trainium2 NeuronCores via Bass/Tile.

Strategy: every pre-FFT step (framing -> DC removal -> preemphasis -> Povey
window) is linear in the frame, so the whole frame->spectrum map folds into
two constant matrices G_re/G_im [400, 256] (Nyquist bin 256 dropped: its mel
weight is exactly zero).  Per frame: power = (f@G_re)^2 + (f@G_im)^2, then
mel = power @ W^T, out = log(max(mel, eps)).  All heavy work is tensor-engine
matmuls in float32r (full-rate fp32 path with ~10-bit-mantissa operands).
To recover fp32 accuracy each product X@G is computed as the 3-term split
Xhi@Ghi + Xlo@Ghi + Xhi@Glo, with lo = exact residual of the f32r rounding
(computed on-chip so it matches the hardware rounding bit-for-bit).

Sharding: batch 32 -> 8 cores x 4 waveforms (embarrassingly data-parallel).
"""

import numpy as np

SR = 16000
WIN = 400
SHIFT = 160
NFFT = 512
NMEL = 80
PREEMPH = 0.97
EPS = 1.1920929e-07

B_FULL = 32
L = 160000
N_CORES = 8
B_CORE = B_FULL // N_CORES          # 4 waveforms per core
M_FRAMES = 1 + (L - WIN) // SHIFT   # 998
NJ = L // SHIFT                     # 1000 blocks of 160 samples
NFREQ = 256                         # bins 0..255 (bin 256 has zero mel weight)

# frame blocks (moving-operand N per matmul; fp32 max is 512)
FRAME_BLOCKS = [(0, 512), (512, M_FRAMES - 512)]
# K chunking of the 400-sample window into 4 partition chunks
K_SIZES = [128, 128, 128, 16]


def _build_consts():
    """G_re/G_im [400, 256] and mel weights [256, 80], fp64 math -> fp32."""
    t = np.arange(WIN, dtype=np.float64)
    povey = (0.5 - 0.5 * np.cos(2.0 * np.pi * t / (WIN - 1))) ** 0.85
    M1 = np.eye(WIN) - np.ones((WIN, WIN)) / WIN      # remove_dc_offset
    P = np.eye(WIN)
    P[0, 0] = 1.0 - PREEMPH                            # preemphasis (replicate pad)
    for i in range(1, WIN):
        P[i, i - 1] = -PREEMPH
    A = povey[:, None] * (P @ M1)                      # [400, 400] combined linear map
    u = np.arange(WIN)[:, None]
    k = np.arange(NFREQ)[None, :]
    ang = 2.0 * np.pi * u * k / NFFT
    G_re = (A.T @ np.cos(ang)).astype(np.float32)      # [400, 256]
    G_im = (A.T @ -np.sin(ang)).astype(np.float32)

    def mel(f):
        return 1127.0 * np.log(1.0 + f / 700.0)

    fft_freqs = np.arange(NFFT // 2) * (SR / NFFT)
    m = mel(fft_freqs)
    ml, mh = mel(20.0), mel(8000.0)
    d = (mh - ml) / (NMEL + 1)
    left = ml + np.arange(NMEL)[:, None] * d
    center = left + d
    right = center + d
    w = np.maximum(0.0, np.minimum((m - left) / (center - left),
                                   (right - m) / (right - center)))  # [80, 256]
    MELW_T = np.ascontiguousarray(w.T).astype(np.float32)            # [256, 80]
    return G_re, G_im, MELW_T


def _build_bass():
    import concourse.mybir as mybir
    from concourse import bacc
    from concourse.masks import make_identity
    from concourse.tile import TileContext

    f32 = mybir.dt.float32
    f32r = mybir.dt.float32r

    nc = bacc.Bacc("TRN2", target_bir_lowering=False, debug=False,
                   num_devices=N_CORES)
    waves = nc.dram_tensor("waves", [B_CORE, L], f32, kind="ExternalInput").ap()
    gre_d = nc.dram_tensor("gre", [WIN, NFREQ], f32, kind="ExternalInput").ap()
    gim_d = nc.dram_tensor("gim", [WIN, NFREQ], f32, kind="ExternalInput").ap()
    melw_d = nc.dram_tensor("melw", [NFREQ, NMEL], f32, kind="ExternalInput").ap()
    out_d = nc.dram_tensor("out", [B_CORE, NMEL, M_FRAMES], f32,
                           kind="ExternalOutput").ap()

    with TileContext(nc) as tc:
        with (
            tc.tile_pool(name="consts", bufs=1) as cpool,
            tc.tile_pool(name="stage", bufs=2) as stpool,
            tc.tile_pool(name="w160", bufs=2) as wpool,
            tc.tile_pool(name="vload", bufs=3) as vpool,
            tc.tile_pool(name="work", bufs=2) as spool,
            tc.tile_pool(name="psum_t", bufs=2, space="PSUM") as pt,
            tc.tile_pool(name="psum_d", bufs=2, space="PSUM") as pd,
            tc.tile_pool(name="psum_m", bufs=2, space="PSUM") as pm,
        ):
            # ---- constants ----
            ident = cpool.tile([128, 128], f32, tag="ident")
            make_identity(nc, ident[:])

            # lhsT K-chunk tiles, hi (f32r-rounded) + lo (exact residual,
            # itself f32r — its own rounding is ~2^-20 relative).  walrus
            # requires every producer feeding an FP32R matmul to round to
            # f32r, hence DMA to an fp32 staging tile + ACT copy (rounds)
            # + DVE subtract for the residual.
            # q3 (K=16) must share the rhs base partition 64, so its tiles
            # are [80, .] with data in rows 64:80.
            def load_split(dst_hi, dst_lo, st, src_ap):
                nc.sync.dma_start(out=st, in_=src_ap)
                nc.scalar.copy(out=dst_hi, in_=st)
                nc.vector.tensor_sub(dst_lo, st, dst_hi)

            ghi, glo = {}, {}
            r0 = 0
            for q, ks in enumerate(K_SIZES):
                for nm, src in (("re", gre_d), ("im", gim_d)):
                    if q < 3:
                        thi = cpool.tile([ks, NFREQ], f32r, tag=f"ghi{nm}{q}")
                        tlo = cpool.tile([ks, NFREQ], f32r, tag=f"glo{nm}{q}")
                        st = stpool.tile([ks, NFREQ], f32, tag="stage")
                        load_split(thi[:], tlo[:], st[:], src[r0:r0 + ks, :])
                        ghi[nm, q] = thi
                        glo[nm, q] = tlo
                    else:
                        thi = cpool.tile([80, NFREQ], f32r, tag=f"ghi{nm}{q}")
                        tlo = cpool.tile([80, NFREQ], f32r, tag=f"glo{nm}{q}")
                        st = stpool.tile([80, NFREQ], f32, tag="stage")
                        load_split(thi[64:80, :], tlo[64:80, :], st[64:80, :],
                                   src[r0:r0 + ks, :])
                        ghi[nm, q] = thi[64:80, :]
                        glo[nm, q] = tlo[64:80, :]
                r0 += ks

            mw_hi, mw_lo = [], []
            for c in range(2):
                whi = cpool.tile([128, NMEL], f32r, tag=f"mwhi{c}")
                wlo = cpool.tile([128, NMEL], f32r, tag=f"mwlo{c}")
                st = stpool.tile([128, NMEL], f32, tag="stage_m")
                nc.sync.dma_start(out=st[:], in_=melw_d[c * 128:(c + 1) * 128, :])
                nc.scalar.copy(out=whi[:], in_=st[:])
                nc.vector.tensor_sub(wlo[:], st[:], whi[:])
                mw_hi.append(whi)
                mw_lo.append(wlo)

            for b in range(B_CORE):
                wav_js = waves[b].rearrange("(j s) -> j s", s=SHIFT)  # [1000, 160]

                # ---- phase T: build W160[s, j] = wave[160 j + s], hi + lo ----
                wtop = wpool.tile([128, NJ], f32r, tag="wtop")   # s in [0, 128)
                wbot = wpool.tile([32, NJ], f32r, tag="wbot")    # s in [128, 160)
                ltop = wpool.tile([128, NJ], f32r, tag="ltop")
                lbot = wpool.tile([32, NJ], f32r, tag="lbot")
                for c in range(8):
                    j0 = c * 128
                    p_c = min(128, NJ - j0)                      # 128 or 104
                    v = vpool.tile([p_c, SHIFT], f32, tag="v")
                    nc.sync.dma_start(out=v[:], in_=wav_js[j0:j0 + p_c, :])
                    tp0 = pt.tile([128, p_c], f32, tag="tp")
                    nc.tensor.transpose(tp0[:], v[:, 0:128], ident[:p_c, :p_c])
                    nc.scalar.copy(out=wtop[:, j0:j0 + p_c], in_=tp0[:])
                    nc.vector.tensor_sub(ltop[:, j0:j0 + p_c], tp0[:],
                                         wtop[:, j0:j0 + p_c])
                    tp1 = pt.tile([32, p_c], f32, tag="tp")
                    nc.tensor.transpose(tp1[:], v[:, 128:160], ident[:p_c, :p_c])
                    nc.scalar.copy(out=wbot[:, j0:j0 + p_c], in_=tp1[:])
                    nc.vector.tensor_sub(lbot[:, j0:j0 + p_c], tp1[:],
                                         wbot[:, j0:j0 + p_c])

                # K-chunk rhs tensors.  t in [128,256): rows 0:32 = wbot@j,
                # rows 32:128 = wtop[0:96]@(j+1).  t in [256,384): wtop[96:128]@(j+1),
                # wbot[0:32]@(j+1), wtop[0:64]@(j+2).  t in [384,400): wtop[64:80]@(j+2).
                # (gpsimd APs must be 32-aligned 32/64/128-partition blocks)
                def assemble(top, bot, tagp):
                    mid = wpool.tile([128, NJ], f32r, tag=f"{tagp}mid")
                    nc.gpsimd.tensor_copy(mid[0:32, 0:NJ], bot[0:32, 0:NJ])
                    nc.gpsimd.tensor_copy(mid[32:64, 0:NJ - 1], top[0:32, 1:NJ])
                    nc.gpsimd.tensor_copy(mid[64:96, 0:NJ - 1], top[32:64, 1:NJ])
                    nc.gpsimd.tensor_copy(mid[96:128, 0:NJ - 1], top[64:96, 1:NJ])
                    mid2 = wpool.tile([128, NJ], f32r, tag=f"{tagp}mid2")
                    nc.gpsimd.tensor_copy(mid2[0:32, 0:NJ - 1], top[96:128, 1:NJ])
                    nc.gpsimd.tensor_copy(mid2[32:64, 0:NJ - 1], bot[0:32, 1:NJ])
                    nc.gpsimd.tensor_copy(mid2[64:96, 0:NJ - 2], top[0:32, 2:NJ])
                    nc.gpsimd.tensor_copy(mid2[96:128, 0:NJ - 2], top[32:64, 2:NJ])
                    return mid, mid2

                wmid, wmid2 = assemble(wtop, wbot, "w")
                lmid, lmid2 = assemble(ltop, lbot, "l")

                # ---- phases D + M per frame block ----
                for (i0, nfb) in FRAME_BLOCKS:
                    def views(top, mid, mid2):
                        return [
                            top[0:128, i0:i0 + nfb],
                            mid[0:128, i0:i0 + nfb],
                            mid2[0:128, i0:i0 + nfb],
                            top[64:80, i0 + 2:i0 + 2 + nfb],
                        ]
                    rhs_hi = views(wtop, wmid, wmid2)
                    rhs_lo = views(ltop, lmid, lmid2)

                    power_hi, power_lo = [], []
                    for mi in range(2):
                        msl = slice(mi * 128, (mi + 1) * 128)
                        sqs = []
                        for nm in ("re", "im"):
                            ps = pd.tile([128, nfb], f32, tag=f"ps_{nm}")
                            for q in range(4):
                                nc.tensor.matmul(
                                    ps[:], ghi[nm, q][:, msl], rhs_hi[q],
                                    start=(q == 0), stop=False)
                            for q in range(4):
                                nc.tensor.matmul(
                                    ps[:], ghi[nm, q][:, msl], rhs_lo[q],
                                    start=False, stop=False)
                            for q in range(4):
                                nc.tensor.matmul(
                                    ps[:], glo[nm, q][:, msl], rhs_hi[q],
                                    start=False, stop=(q == 3))
                            sq = spool.tile([128, nfb], f32, tag=f"sq_{nm}")
                            nc.scalar.square(sq[:], ps[:])
                            sqs.append(sq)
                        pw32 = spool.tile([128, nfb], f32, tag="pw32")
                        nc.vector.tensor_add(pw32[:], sqs[0][:], sqs[1][:])
                        phi = spool.tile([128, nfb], f32r, tag="phi")
                        nc.gpsimd.tensor_copy(phi[:], pw32[:])
                        plo = spool.tile([128, nfb], f32r, tag="plo")
                        nc.vector.tensor_sub(plo[:], pw32[:], phi[:])
                        power_hi.append(phi)
                        power_lo.append(plo)

                    ps_mel = pm.tile([NMEL, nfb], f32, tag="mel")
                    for mi in range(2):
                        nc.tensor.matmul(ps_mel[:], mw_hi[mi][:], power_hi[mi][:],
                                         start=(mi == 0), stop=False)
                        nc.tensor.matmul(ps_mel[:], mw_hi[mi][:], power_lo[mi][:],
                                         start=False, stop=False)
                        nc.tensor.matmul(ps_mel[:], mw_lo[mi][:], power_hi[mi][:],
                                         start=False, stop=(mi == 1))
                    mel_sb = spool.tile([NMEL, nfb], f32, tag="mel_sb")
                    nc.vector.tensor_scalar_max(mel_sb[:], ps_mel[:], EPS)
                    out_sb = spool.tile([NMEL, nfb], f32, tag="out_sb")
                    nc.scalar.activation(out_sb[:], mel_sb[:],
                                         mybir.ActivationFunctionType.Ln)
                    nc.sync.dma_start(out=out_d[b][:, i0:i0 + nfb], in_=out_sb[:])

    nc.compile()
    return nc


_CACHE = {}


def kernel(waveforms) -> np.ndarray:
    from concourse.bass_utils import run_bass_kernel_spmd

    w = np.ascontiguousarray(np.asarray(waveforms, dtype=np.float32))
    assert w.shape == (B_FULL, L), w.shape

    if "nc" not in _CACHE:
        _CACHE["consts"] = _build_consts()
        _CACHE["nc"] = _build_bass()
    G_re, G_im, MELW_T = _CACHE["consts"]
    nc = _CACHE["nc"]

    shards = w.reshape(N_CORES, B_CORE, L)
    in_maps = [
        {"waves": shards[c], "gre": G_re, "gim": G_im, "melw": MELW_T}
        for c in range(N_CORES)
    ]
    r = run_bass_kernel_spmd(nc, in_maps, list(range(N_CORES)))
    out = np.concatenate([r.results[c]["out"] for c in range(N_CORES)], axis=0)
    return out
